# revision 8
# baseline (speedup 1.0000x reference)
"""Trainium2 Bass kernel for CustomGraphConv message passing.

Computation (per reference):
    msg_e   = einsum('a,aoi,i->o', edge_attr[e], W, x[src_e])     [E, 16]
    aggr    = segment_sum(msg, dst, num_nodes)                    [N, 16]
    out     = relu(aggr + bias)

Device strategy (8 cores, no collectives):
  * Shard by DESTINATION node range: core k owns nodes [k*12544, (k+1)*12544)
    and exactly the edges pointing into that range.  Output slices are
    disjoint -> no all-reduce; the host just concatenates.
  * Host sorts edges by dst and pads each 128-node "group" run to a
    multiple of 128 edges (dummy edges with edge_attr=0 contribute 0).
  * Per 128-edge chunk on device:
      - gather x[src] rows via indirect DMA        -> xj   [128e, 16]
      - z = outer(edge_attr_e, xj_e)  (DVE bcast)  -> z    [128e, 128(a,i)]
      - onehot[e, n] = (dst_local[e] == n)         -> oh   [128e, 128n]
      - PSUM accumulate  Q_T += z.T @ oh           -> Q_T  [128(a,i), 128n]
    Then per group:  aggr = (Q_T).T @ W2  ([128n, 16]), + bias, relu, DMA out.
    where W2[(a,i), o] = W[a, o, i] so that msg = z @ W2.
"""

import math
import os

import numpy as np

P = 128          # SBUF partitions == edges per chunk == nodes per group
A = 8            # edge-attr width
CIN = 16         # input channels
COUT = 16        # output channels


class Cfg:
    def __init__(self, n_nodes, n_edges, n_cores=8, groups_per_core=98,
                 slab_groups=14, cpg=None):
        self.n_nodes = n_nodes
        self.n_edges = n_edges
        self.n_cores = n_cores
        self.gpc = groups_per_core          # node groups per core
        self.npc = P * self.gpc             # nodes per core (padded)
        assert self.npc * n_cores >= n_nodes
        self.slab_groups = slab_groups      # groups per DMA slab
        assert self.gpc % slab_groups == 0
        self.nslabs = self.gpc // slab_groups
        self.cpg = cpg                      # chunks per group (data dependent)

    @property
    def sb_chunks(self):                    # chunks per slab
        return self.slab_groups * self.cpg


FULL = Cfg(n_nodes=100000, n_edges=1600000)

F32 = None  # filled lazily (mybir import)


# --------------------------------------------------------------------------
# host-side sharding / layout
# --------------------------------------------------------------------------

def host_prep(cfg, x, edge_index, edge_attr, weight_matrix, bias):
    """Sort by dst, pad group runs to 128-edge chunks, build per-core arrays."""
    src = np.asarray(edge_index[0]).astype(np.int64)
    dst = np.asarray(edge_index[1]).astype(np.int64)
    ea = np.ascontiguousarray(np.asarray(edge_attr), dtype=np.float32)
    x = np.ascontiguousarray(np.asarray(x), dtype=np.float32)

    perm = np.argsort(dst, kind="stable")
    srcs = src[perm].astype(np.int32)
    dsts = dst[perm].astype(np.int32)
    eas = ea[perm]

    n_groups = cfg.gpc * cfg.n_cores
    g = dsts >> 7                                  # dst // 128
    counts = np.bincount(g, minlength=n_groups)
    cpg = max(1, int(math.ceil(counts.max() / P)))
    if cfg.cpg is None:
        cfg.cpg = cpg
    else:
        assert cpg <= cfg.cpg, "data needs more chunks per group than compiled"
    B = cfg.cpg

    slots_pg = B * P
    gstart = np.zeros(n_groups + 1, np.int64)
    gstart[1:] = np.cumsum(counts)
    rank = np.arange(len(dsts), dtype=np.int64) - gstart[g]
    slot = g.astype(np.int64) * slots_pg + rank

    tot = n_groups * slots_pg
    srcp = np.zeros(tot, np.int32)
    eap = np.zeros((tot, A), np.float32)
    dstl = np.zeros(tot, np.int32)
    srcp[slot] = srcs
    eap[slot] = eas
    dstl[slot] = dsts & (P - 1)

    NC, GPC, SLAB, NS = cfg.n_cores, cfg.gpc, cfg.slab_groups, cfg.nslabs
    # [cores, slabs, slab_groups, B, P(, A)] ; edge identity = (group, chunk, part)
    srcp = srcp.reshape(NC, NS, SLAB, B, P)
    dstl = dstl.reshape(NC, NS, SLAB, B, P)
    eap = eap.reshape(NC, NS, SLAB, B, P, A)
    # device slab layouts: partition-major, per-partition contiguous free dim
    idx_host = np.ascontiguousarray(srcp.transpose(0, 1, 4, 2, 3)) \
        .reshape(NC, NS, P, SLAB * B)                       # int32
    dst_host = np.ascontiguousarray(dstl.transpose(0, 1, 4, 2, 3)) \
        .reshape(NC, NS, P, SLAB * B).astype(np.float32)
    ea_host = np.ascontiguousarray(eap.transpose(0, 1, 4, 2, 3, 5)) \
        .reshape(NC, NS, P, SLAB * B * A)                   # f32

    w2 = np.ascontiguousarray(
        np.asarray(weight_matrix, dtype=np.float32).transpose(0, 2, 1)
    ).reshape(A * CIN, COUT)                                # [(a,i), o]
    bias_t = np.ascontiguousarray(
        np.broadcast_to(np.asarray(bias, dtype=np.float32).reshape(1, COUT),
                        (P, COUT))
    )
    return x, idx_host, dst_host, ea_host, w2, bias_t


# --------------------------------------------------------------------------
# device kernel
# --------------------------------------------------------------------------

def build_bass(cfg, use_bf16=True):
    import concourse.bacc as bacc
    import concourse.bass as bass
    import concourse.mybir as mybir
    import concourse.tile as tile
    from concourse._compat import axon_active

    f32 = mybir.dt.float32
    bf16 = mybir.dt.bfloat16
    i32 = mybir.dt.int32
    cdt = bf16 if use_bf16 else f32   # compute dtype for z / onehot / matmul

    B = cfg.cpg
    SB = cfg.sb_chunks      # chunks per slab
    SLAB = cfg.slab_groups

    nc = bacc.Bacc(
        "TRN2",
        target_bir_lowering=False,
        debug=False,
        enable_asserts=False,
        num_devices=cfg.n_cores,
    )

    x_d = nc.dram_tensor("x", [cfg.n_nodes, CIN], f32, kind="ExternalInput")
    ea_d = nc.dram_tensor("ea", [cfg.nslabs, P, SB * A], f32, kind="ExternalInput")
    idx_d = nc.dram_tensor("idx", [cfg.nslabs, P, SB], i32, kind="ExternalInput")
    dst_d = nc.dram_tensor("dstl", [cfg.nslabs, P, SB], f32, kind="ExternalInput")
    w2_d = nc.dram_tensor("w2", [A * CIN, COUT], f32, kind="ExternalInput")
    b_d = nc.dram_tensor("bias", [P, COUT], f32, kind="ExternalInput")
    out_d = nc.dram_tensor(
        "out", [cfg.nslabs, SLAB, P, COUT], f32, kind="ExternalOutput"
    )

    with tile.TileContext(nc) as tc:
        with (
            tc.tile_pool(name="const", bufs=1) as cpool,
            tc.tile_pool(name="slab_in", bufs=2) as spool,
            tc.tile_pool(name="xj", bufs=2) as xjpool,
            tc.tile_pool(name="zoh", bufs=3) as zpool,
            tc.tile_pool(name="q", bufs=2) as qpool,
            tc.tile_pool(name="ostage", bufs=2) as opool,
            tc.tile_pool(name="psq", bufs=3, space="PSUM") as psq,
            tc.tile_pool(name="pso", bufs=2, space="PSUM") as pso,
        ):
            # constants
            iota_t = cpool.tile([P, P], cdt, tag="iota")
            nc.gpsimd.iota(iota_t[:], pattern=[[1, P]], base=0,
                           channel_multiplier=0,
                           allow_small_or_imprecise_dtypes=True)
            w2_t = cpool.tile([A * CIN, COUT], f32, tag="w2")
            nc.sync.dma_start(out=w2_t[:], in_=w2_d.ap())
            bias_t = cpool.tile([P, COUT], f32, tag="bias")
            nc.sync.dma_start(out=bias_t[:], in_=b_d.ap())

            for s in range(cfg.nslabs):
                idx_t = spool.tile([P, SB], i32, tag="idx")
                nc.sync.dma_start(out=idx_t[:], in_=idx_d.ap()[s])
                dst_t = spool.tile([P, SB], cdt, tag="dst")
                if use_bf16:
                    # values are 0..127: exact in bf16; SWDGE casts in flight
                    nc.gpsimd.dma_start(out=dst_t[:], in_=dst_d.ap()[s])
                else:
                    nc.sync.dma_start(out=dst_t[:], in_=dst_d.ap()[s])
                if use_bf16:
                    ea_t = spool.tile([P, SB * A], bf16, tag="ea")
                    # SWDGE casts f32 -> bf16 during the copy
                    nc.gpsimd.dma_start(out=ea_t[:], in_=ea_d.ap()[s])
                else:
                    ea_t = spool.tile([P, SB * A], f32, tag="ea")
                    nc.sync.dma_start(out=ea_t[:], in_=ea_d.ap()[s])

                # indirect gather: HW consumes exactly ONE index per
                # partition per instruction (per-partition dynamic block
                # copy), so issue one gather per 128-edge chunk.
                xj_t = xjpool.tile([P, SB * CIN], f32, tag="xj")
                for c in range(SB):
                    nc.gpsimd.indirect_dma_start(
                        out=xj_t[:, c * CIN:(c + 1) * CIN],
                        out_offset=None,
                        in_=x_d.ap(),
                        in_offset=bass.IndirectOffsetOnAxis(
                            ap=idx_t[:, c:c + 1], axis=0),
                    )
                if use_bf16:
                    xjc_t = xjpool.tile([P, SB * CIN], bf16, tag="xjc")
                    nc.vector.tensor_copy(out=xjc_t[:], in_=xj_t[:])
                else:
                    xjc_t = xj_t

                out_sb = opool.tile([P, SLAB * COUT], f32, tag="ostage")

                for gs in range(SLAB):
                    # z[e, (c, a, i)] = ea[e, c, a] * xj[e, c, i]
                    z_t = zpool.tile([P, B * P], cdt, tag="z")
                    ea_ap = (
                        ea_t[:, gs * B * A:(gs + 1) * B * A]
                        .rearrange("p (b a) -> p b a", a=A)
                        .unsqueeze(3)
                        .to_broadcast([P, B, A, CIN])
                    )
                    xj_ap = (
                        xjc_t[:, gs * B * CIN:(gs + 1) * B * CIN]
                        .rearrange("p (b i) -> p b i", i=CIN)
                        .unsqueeze(2)
                        .to_broadcast([P, B, A, CIN])
                    )
                    z_ap = z_t[:].rearrange("p (b a i) -> p b a i", a=A, i=CIN)
                    nc.vector.tensor_tensor(
                        out=z_ap, in0=ea_ap, in1=xj_ap, op=mybir.AluOpType.mult
                    )

                    # onehot[e, (c, n)] = (dst_local[e, c] == n)
                    oh_t = zpool.tile([P, B * P], cdt, tag="oh")
                    iota_ap = (
                        iota_t[:].unsqueeze(1).to_broadcast([P, B, P])
                    )
                    dstg_ap = (
                        dst_t[:, gs * B:(gs + 1) * B]
                        .unsqueeze(2)
                        .to_broadcast([P, B, P])
                    )
                    oh_ap = oh_t[:].rearrange("p (b n) -> p b n", n=P)
                    nc.vector.tensor_tensor(
                        out=oh_ap, in0=iota_ap, in1=dstg_ap,
                        op=mybir.AluOpType.is_equal,
                    )

                    # Q_T[(a,i), n] += z.T @ onehot     (accumulate B chunks)
                    q_ps = psq.tile([P, P], f32, tag="qps")
                    for c in range(B):
                        nc.tensor.matmul(
                            out=q_ps[:],
                            lhsT=z_t[:, c * P:(c + 1) * P],
                            rhs=oh_t[:, c * P:(c + 1) * P],
                            start=(c == 0),
                            stop=(c == B - 1),
                        )
                    q_sb = qpool.tile([P, P], f32, tag="qsb")
                    nc.scalar.activation(
                        out=q_sb[:], in_=q_ps[:],
                        func=mybir.ActivationFunctionType.Copy,
                    )

                    # aggr = Q_T.T @ W2   -> [128n, 16]
                    o_ps = pso.tile([P, COUT], f32, tag="ops")
                    nc.tensor.matmul(
                        out=o_ps[:], lhsT=q_sb[:], rhs=w2_t[:],
                        start=True, stop=True,
                    )
                    # relu(aggr + bias): add bias, then clamp at 0 in place
                    oslice = out_sb[:, gs * COUT:(gs + 1) * COUT]
                    nc.vector.tensor_tensor(
                        out=oslice, in0=o_ps[:], in1=bias_t[:],
                        op=mybir.AluOpType.add,
                    )
                    nc.vector.tensor_scalar(
                        out=oslice, in0=oslice, scalar1=0.0, scalar2=None,
                        op0=mybir.AluOpType.max,
                    )

                # store the slab: SBUF [128, SLAB*16] -> DRAM [SLAB, 128, 16]
                nc.sync.dma_start(
                    out=out_d.ap()[s].transpose([1, 0, 2]),
                    in_=out_sb[:].rearrange("p (g o) -> p g o", o=COUT),
                )

    nc.compile()
    return nc


# --------------------------------------------------------------------------
# runner
# --------------------------------------------------------------------------

_CACHE = {}


def _get_compiled(cfg, use_bf16):
    key = (cfg.n_nodes, cfg.n_edges, cfg.cpg, use_bf16)
    if key not in _CACHE:
        _CACHE[key] = build_bass(cfg, use_bf16=use_bf16)
    return _CACHE[key]


def kernel(x, edge_index, edge_attr, weight_matrix, bias, num_nodes):
    cfg = Cfg(n_nodes=FULL.n_nodes, n_edges=FULL.n_edges)
    assert int(num_nodes) == cfg.n_nodes
    use_bf16 = os.environ.get("GNN_BF16", "0") == "1"

    xf, idx_h, dst_h, ea_h, w2, bias_f = host_prep(
        cfg, x, edge_index, edge_attr, weight_matrix, bias
    )
    nc = _get_compiled(cfg, use_bf16)

    in_maps = []
    for k in range(cfg.n_cores):
        in_maps.append({
            "x": xf,
            "ea": ea_h[k],
            "idx": idx_h[k],
            "dstl": dst_h[k],
            "w2": w2,
            "bias": bias_f,
        })

    from concourse import bass_utils
    res = bass_utils.run_bass_kernel_spmd(
        nc, in_maps, core_ids=list(range(cfg.n_cores)),
        trace=os.environ.get("GNN_TRACE", "0") == "1",
    )
    outs = [r["out"].reshape(-1, COUT) for r in res.results]
    full = np.concatenate(outs, axis=0)[:cfg.n_nodes]
    kernel.last_results = res
    return np.ascontiguousarray(full, dtype=np.float32)


kernel.last_results = None


# revision 10
# speedup vs baseline: 1.0081x; 1.0081x over previous
"""Trainium2 Bass kernel for CustomGraphConv message passing.

Computation (per reference):
    msg_e   = einsum('a,aoi,i->o', edge_attr[e], W, x[src_e])     [E, 16]
    aggr    = segment_sum(msg, dst, num_nodes)                    [N, 16]
    out     = relu(aggr + bias)

Device strategy (8 cores, no collectives):
  * Shard by DESTINATION node range: core k owns nodes [k*12544, (k+1)*12544)
    and exactly the edges pointing into that range.  Output slices are
    disjoint -> no all-reduce; the host just concatenates.
  * Host sorts edges by dst and pads each 128-node "group" run to a
    multiple of 128 edges (dummy edges with edge_attr=0 contribute 0).
  * Per 128-edge chunk on device:
      - gather x[src] rows via indirect DMA        -> xj   [128e, 16]
      - z = outer(edge_attr_e, xj_e)  (DVE bcast)  -> z    [128e, 128(a,i)]
      - onehot[e, n] = (dst_local[e] == n)         -> oh   [128e, 128n]
      - PSUM accumulate  Q_T += z.T @ oh           -> Q_T  [128(a,i), 128n]
    Then per group:  aggr = (Q_T).T @ W2  ([128n, 16]), + bias, relu, DMA out.
    where W2[(a,i), o] = W[a, o, i] so that msg = z @ W2.
"""

import math
import os

import numpy as np

P = 128          # SBUF partitions == edges per chunk == nodes per group
A = 8            # edge-attr width
CIN = 16         # input channels
COUT = 16        # output channels


class Cfg:
    def __init__(self, n_nodes, n_edges, n_cores=8, groups_per_core=98,
                 slab_groups=14, cpg=None):
        self.n_nodes = n_nodes
        self.n_edges = n_edges
        self.n_cores = n_cores
        self.gpc = groups_per_core          # node groups per core
        self.npc = P * self.gpc             # nodes per core (padded)
        assert self.npc * n_cores >= n_nodes
        self.slab_groups = slab_groups      # groups per DMA slab
        assert self.gpc % slab_groups == 0
        self.nslabs = self.gpc // slab_groups
        self.cpg = cpg                      # chunks per group (data dependent)

    @property
    def sb_chunks(self):                    # chunks per slab
        return self.slab_groups * self.cpg


FULL = Cfg(n_nodes=100000, n_edges=1600000)

F32 = None  # filled lazily (mybir import)


# --------------------------------------------------------------------------
# host-side sharding / layout
# --------------------------------------------------------------------------

def host_prep(cfg, x, edge_index, edge_attr, weight_matrix, bias):
    """Sort by dst, pad group runs to 128-edge chunks, build per-core arrays."""
    src = np.asarray(edge_index[0]).astype(np.int64)
    dst = np.asarray(edge_index[1]).astype(np.int64)
    ea = np.ascontiguousarray(np.asarray(edge_attr), dtype=np.float32)
    x = np.ascontiguousarray(np.asarray(x), dtype=np.float32)

    perm = np.argsort(dst, kind="stable")
    srcs = src[perm].astype(np.int32)
    dsts = dst[perm].astype(np.int32)
    eas = ea[perm]

    n_groups = cfg.gpc * cfg.n_cores
    g = dsts >> 7                                  # dst // 128
    counts = np.bincount(g, minlength=n_groups)
    cpg = max(1, int(math.ceil(counts.max() / P)))
    if cfg.cpg is None:
        cfg.cpg = cpg
    else:
        assert cpg <= cfg.cpg, "data needs more chunks per group than compiled"
    B = cfg.cpg

    slots_pg = B * P
    gstart = np.zeros(n_groups + 1, np.int64)
    gstart[1:] = np.cumsum(counts)
    rank = np.arange(len(dsts), dtype=np.int64) - gstart[g]
    slot = g.astype(np.int64) * slots_pg + rank

    tot = n_groups * slots_pg
    srcp = np.zeros(tot, np.int32)
    eap = np.zeros((tot, A), np.float32)
    dstl = np.zeros(tot, np.int32)
    srcp[slot] = srcs
    eap[slot] = eas
    dstl[slot] = dsts & (P - 1)

    NC, GPC, SLAB, NS = cfg.n_cores, cfg.gpc, cfg.slab_groups, cfg.nslabs
    # [cores, slabs, slab_groups, B, P(, A)] ; edge identity = (group, chunk, part)
    srcp = srcp.reshape(NC, NS, SLAB, B, P)
    dstl = dstl.reshape(NC, NS, SLAB, B, P)
    eap = eap.reshape(NC, NS, SLAB, B, P, A)
    # device slab layouts: partition-major, per-partition contiguous free dim
    idx_host = np.ascontiguousarray(srcp.transpose(0, 1, 4, 2, 3)) \
        .reshape(NC, NS, P, SLAB * B)                       # int32
    dst_host = np.ascontiguousarray(dstl.transpose(0, 1, 4, 2, 3)) \
        .reshape(NC, NS, P, SLAB * B).astype(np.float32)
    ea_host = np.ascontiguousarray(eap.transpose(0, 1, 4, 2, 3, 5)) \
        .reshape(NC, NS, P, SLAB * B * A)                   # f32

    w2 = np.ascontiguousarray(
        np.asarray(weight_matrix, dtype=np.float32).transpose(0, 2, 1)
    ).reshape(A * CIN, COUT)                                # [(a,i), o]
    bias_t = np.ascontiguousarray(
        np.broadcast_to(np.asarray(bias, dtype=np.float32).reshape(1, COUT),
                        (P, COUT))
    )
    return x, idx_host, dst_host, ea_host, w2, bias_t


# --------------------------------------------------------------------------
# device kernel
# --------------------------------------------------------------------------

def build_bass(cfg, use_bf16=True):
    import concourse.bacc as bacc
    import concourse.bass as bass
    import concourse.mybir as mybir
    import concourse.tile as tile
    from concourse._compat import axon_active

    f32 = mybir.dt.float32
    bf16 = mybir.dt.bfloat16
    i32 = mybir.dt.int32
    cdt = bf16 if use_bf16 else f32   # compute dtype for z / onehot / matmul

    B = cfg.cpg
    SB = cfg.sb_chunks      # chunks per slab
    SLAB = cfg.slab_groups

    nc = bacc.Bacc(
        "TRN2",
        target_bir_lowering=False,
        debug=False,
        enable_asserts=False,
        num_devices=cfg.n_cores,
    )

    x_d = nc.dram_tensor("x", [cfg.n_nodes, CIN], f32, kind="ExternalInput")
    ea_d = nc.dram_tensor("ea", [cfg.nslabs, P, SB * A], f32, kind="ExternalInput")
    idx_d = nc.dram_tensor("idx", [cfg.nslabs, P, SB], i32, kind="ExternalInput")
    dst_d = nc.dram_tensor("dstl", [cfg.nslabs, P, SB], f32, kind="ExternalInput")
    w2_d = nc.dram_tensor("w2", [A * CIN, COUT], f32, kind="ExternalInput")
    b_d = nc.dram_tensor("bias", [P, COUT], f32, kind="ExternalInput")
    out_d = nc.dram_tensor(
        "out", [cfg.nslabs, SLAB, P, COUT], f32, kind="ExternalOutput"
    )

    with tile.TileContext(nc) as tc:
        with (
            tc.tile_pool(name="const", bufs=1) as cpool,
            tc.tile_pool(name="slab_in", bufs=2) as spool,
            tc.tile_pool(name="xj", bufs=2) as xjpool,
            tc.tile_pool(name="zoh", bufs=3) as zpool,
            tc.tile_pool(name="q", bufs=2) as qpool,
            tc.tile_pool(name="ostage", bufs=2) as opool,
            tc.tile_pool(name="psq", bufs=3, space="PSUM") as psq,
            tc.tile_pool(name="pso", bufs=2, space="PSUM") as pso,
        ):
            # constants
            iota_t = cpool.tile([P, P], cdt, tag="iota")
            nc.gpsimd.iota(iota_t[:], pattern=[[1, P]], base=0,
                           channel_multiplier=0,
                           allow_small_or_imprecise_dtypes=True)
            w2_t = cpool.tile([A * CIN, COUT], f32, tag="w2")
            nc.sync.dma_start(out=w2_t[:], in_=w2_d.ap())
            bias_t = cpool.tile([P, COUT], f32, tag="bias")
            nc.sync.dma_start(out=bias_t[:], in_=b_d.ap())

            for s in range(cfg.nslabs):
                idx_t = spool.tile([P, SB], i32, tag="idx")
                nc.sync.dma_start(out=idx_t[:], in_=idx_d.ap()[s])
                dst_t = spool.tile([P, SB], cdt, tag="dst")
                if use_bf16:
                    # values are 0..127: exact in bf16; SWDGE casts in flight
                    nc.gpsimd.dma_start(out=dst_t[:], in_=dst_d.ap()[s])
                else:
                    nc.sync.dma_start(out=dst_t[:], in_=dst_d.ap()[s])
                if use_bf16:
                    ea_t = spool.tile([P, SB * A], bf16, tag="ea")
                    # SWDGE casts f32 -> bf16 during the copy
                    nc.gpsimd.dma_start(out=ea_t[:], in_=ea_d.ap()[s])
                else:
                    ea_t = spool.tile([P, SB * A], f32, tag="ea")
                    nc.sync.dma_start(out=ea_t[:], in_=ea_d.ap()[s])

                # indirect gather: HW consumes exactly ONE index per
                # partition per instruction (per-partition dynamic block
                # copy), so issue one gather per 128-edge chunk.
                xj_t = xjpool.tile([P, SB * CIN], f32, tag="xj")
                for c in range(SB):
                    nc.gpsimd.indirect_dma_start(
                        out=xj_t[:, c * CIN:(c + 1) * CIN],
                        out_offset=None,
                        in_=x_d.ap(),
                        in_offset=bass.IndirectOffsetOnAxis(
                            ap=idx_t[:, c:c + 1], axis=0),
                    )
                if use_bf16:
                    xjc_t = xjpool.tile([P, SB * CIN], bf16, tag="xjc")
                    nc.vector.tensor_copy(out=xjc_t[:], in_=xj_t[:])
                else:
                    xjc_t = xj_t

                out_sb = opool.tile([P, SLAB * COUT], f32, tag="ostage")

                for gs in range(SLAB):
                    # z[e, (c, a, i)] = ea[e, c, a] * xj[e, c, i]
                    z_t = zpool.tile([P, B * P], cdt, tag="z")
                    ea_ap = (
                        ea_t[:, gs * B * A:(gs + 1) * B * A]
                        .rearrange("p (b a) -> p b a", a=A)
                        .unsqueeze(3)
                        .to_broadcast([P, B, A, CIN])
                    )
                    xj_ap = (
                        xjc_t[:, gs * B * CIN:(gs + 1) * B * CIN]
                        .rearrange("p (b i) -> p b i", i=CIN)
                        .unsqueeze(2)
                        .to_broadcast([P, B, A, CIN])
                    )
                    z_ap = z_t[:].rearrange("p (b a i) -> p b a i", a=A, i=CIN)
                    nc.vector.tensor_tensor(
                        out=z_ap, in0=ea_ap, in1=xj_ap, op=mybir.AluOpType.mult
                    )

                    # onehot[e, (c, n)] = (dst_local[e, c] == n)
                    oh_t = zpool.tile([P, B * P], cdt, tag="oh")
                    iota_ap = (
                        iota_t[:].unsqueeze(1).to_broadcast([P, B, P])
                    )
                    dstg_ap = (
                        dst_t[:, gs * B:(gs + 1) * B]
                        .unsqueeze(2)
                        .to_broadcast([P, B, P])
                    )
                    oh_ap = oh_t[:].rearrange("p (b n) -> p b n", n=P)
                    nc.vector.tensor_tensor(
                        out=oh_ap, in0=iota_ap, in1=dstg_ap,
                        op=mybir.AluOpType.is_equal,
                    )

                    # Q_T[(a,i), n] += z.T @ onehot     (accumulate B chunks)
                    q_ps = psq.tile([P, P], f32, tag="qps")
                    for c in range(B):
                        nc.tensor.matmul(
                            out=q_ps[:],
                            lhsT=z_t[:, c * P:(c + 1) * P],
                            rhs=oh_t[:, c * P:(c + 1) * P],
                            start=(c == 0),
                            stop=(c == B - 1),
                        )
                    q_sb = qpool.tile([P, P], f32, tag="qsb")
                    nc.scalar.activation(
                        out=q_sb[:], in_=q_ps[:],
                        func=mybir.ActivationFunctionType.Copy,
                    )

                    # aggr = Q_T.T @ W2   -> [128n, 16]
                    o_ps = pso.tile([P, COUT], f32, tag="ops")
                    nc.tensor.matmul(
                        out=o_ps[:], lhsT=q_sb[:], rhs=w2_t[:],
                        start=True, stop=True,
                    )
                    # relu(aggr + bias): add bias, then clamp at 0 in place
                    oslice = out_sb[:, gs * COUT:(gs + 1) * COUT]
                    nc.vector.tensor_tensor(
                        out=oslice, in0=o_ps[:], in1=bias_t[:],
                        op=mybir.AluOpType.add,
                    )
                    nc.vector.tensor_scalar(
                        out=oslice, in0=oslice, scalar1=0.0, scalar2=None,
                        op0=mybir.AluOpType.max,
                    )

                # store the slab: SBUF [128, SLAB*16] -> DRAM [SLAB, 128, 16]
                nc.sync.dma_start(
                    out=out_d.ap()[s].transpose([1, 0, 2]),
                    in_=out_sb[:].rearrange("p (g o) -> p g o", o=COUT),
                )

    nc.compile()
    return nc


# --------------------------------------------------------------------------
# runner
# --------------------------------------------------------------------------

_CACHE = {}


def _get_compiled(cfg, use_bf16):
    key = (cfg.n_nodes, cfg.n_edges, cfg.cpg, use_bf16)
    if key not in _CACHE:
        _CACHE[key] = build_bass(cfg, use_bf16=use_bf16)
    return _CACHE[key]


def kernel(x, edge_index, edge_attr, weight_matrix, bias, num_nodes):
    import time as _time
    cfg = Cfg(n_nodes=FULL.n_nodes, n_edges=FULL.n_edges)
    assert int(num_nodes) == cfg.n_nodes
    use_bf16 = os.environ.get("GNN_BF16", "0") == "1"

    _t0 = _time.time()
    xf, idx_h, dst_h, ea_h, w2, bias_f = host_prep(
        cfg, x, edge_index, edge_attr, weight_matrix, bias
    )
    _t1 = _time.time()
    nc = _get_compiled(cfg, use_bf16)
    _t2 = _time.time()

    in_maps = []
    for k in range(cfg.n_cores):
        in_maps.append({
            "x": xf,
            "ea": ea_h[k],
            "idx": idx_h[k],
            "dstl": dst_h[k],
            "w2": w2,
            "bias": bias_f,
        })

    from concourse import bass_utils
    res = bass_utils.run_bass_kernel_spmd(
        nc, in_maps, core_ids=list(range(cfg.n_cores)),
        trace=os.environ.get("GNN_TRACE", "0") == "1",
    )
    _t3 = _time.time()
    print(f"[kernel] host_prep {_t1 - _t0:.2f}s  compile {_t2 - _t1:.2f}s  "
          f"device-run {_t3 - _t2:.2f}s")
    outs = [r["out"].reshape(-1, COUT) for r in res.results]
    full = np.concatenate(outs, axis=0)[:cfg.n_nodes]
    kernel.last_results = res
    return np.ascontiguousarray(full, dtype=np.float32)


kernel.last_results = None


# revision 12
# speedup vs baseline: 1.0467x; 1.0382x over previous
"""Trainium2 Bass kernel for CustomGraphConv message passing.

Computation (per reference):
    msg_e   = einsum('a,aoi,i->o', edge_attr[e], W, x[src_e])     [E, 16]
    aggr    = segment_sum(msg, dst, num_nodes)                    [N, 16]
    out     = relu(aggr + bias)

Device strategy (8 cores, no collectives):
  * Shard by DESTINATION node range: core k owns nodes [k*12544, (k+1)*12544)
    and exactly the edges pointing into that range.  Output slices are
    disjoint -> no all-reduce; the host just concatenates.
  * Host sorts edges by dst and pads each 128-node "group" run to a
    multiple of 128 edges (dummy edges with edge_attr=0 contribute 0).
  * Per 128-edge chunk on device:
      - gather x[src] rows via indirect DMA        -> xj   [128e, 16]
      - z = outer(edge_attr_e, xj_e)  (DVE bcast)  -> z    [128e, 128(a,i)]
      - onehot[e, n] = (dst_local[e] == n)         -> oh   [128e, 128n]
      - PSUM accumulate  Q_T += z.T @ oh           -> Q_T  [128(a,i), 128n]
    Then per group:  aggr = (Q_T).T @ W2  ([128n, 16]), + bias, relu, DMA out.
    where W2[(a,i), o] = W[a, o, i] so that msg = z @ W2.
"""

import math
import os

import numpy as np

P = 128          # SBUF partitions == edges per chunk == nodes per group
A = 8            # edge-attr width
CIN = 16         # input channels
COUT = 16        # output channels


class Cfg:
    def __init__(self, n_nodes, n_edges, n_cores=8, groups_per_core=98,
                 slab_groups=14, cpg=None):
        self.n_nodes = n_nodes
        self.n_edges = n_edges
        self.n_cores = n_cores
        self.gpc = groups_per_core          # node groups per core
        self.npc = P * self.gpc             # nodes per core (padded)
        assert self.npc * n_cores >= n_nodes
        self.slab_groups = slab_groups      # groups per DMA slab
        assert self.gpc % slab_groups == 0
        self.nslabs = self.gpc // slab_groups
        self.cpg = cpg                      # chunks per group (data dependent)

    @property
    def sb_chunks(self):                    # chunks per slab
        return self.slab_groups * self.cpg


FULL = Cfg(n_nodes=100000, n_edges=1600000)

F32 = None  # filled lazily (mybir import)


# --------------------------------------------------------------------------
# host-side sharding / layout
# --------------------------------------------------------------------------

def host_prep(cfg, x, edge_index, edge_attr, weight_matrix, bias):
    """Sort by dst, pad group runs to 128-edge chunks, build per-core arrays."""
    src = np.asarray(edge_index[0]).astype(np.int64)
    dst = np.asarray(edge_index[1]).astype(np.int64)
    ea = np.ascontiguousarray(np.asarray(edge_attr), dtype=np.float32)
    x = np.ascontiguousarray(np.asarray(x), dtype=np.float32)

    perm = np.argsort(dst, kind="stable")
    dsts = dst[perm].astype(np.int32)

    n_groups = cfg.gpc * cfg.n_cores
    g = dsts >> 7                                  # dst // 128
    counts = np.bincount(g, minlength=n_groups)
    cpg = max(1, int(math.ceil(counts.max() / P)))
    if cfg.cpg is None:
        cfg.cpg = cpg
    else:
        assert cpg <= cfg.cpg, "data needs more chunks per group than compiled"
    B = cfg.cpg

    NC, GPC, SLAB, NS = cfg.n_cores, cfg.gpc, cfg.slab_groups, cfg.nslabs
    gstart = np.zeros(n_groups + 1, np.int64)
    gstart[1:] = np.cumsum(counts)
    rank = np.arange(len(dsts), dtype=np.int64) - gstart[g]
    # scatter straight into the final device layout [NC, NS, P, SLAB*B(*A)]:
    # edge (group g, chunk c, partition p) lands at free pos gs*B + c of
    # partition p in slab ns of core g//GPC.
    c = rank >> 7
    p = rank & (P - 1)
    g64 = g.astype(np.int64)
    core = g64 // GPC
    gi = g64 % GPC
    ns = gi // SLAB
    gs = gi % SLAB
    pos = (((core * NS + ns) * P + p) * SLAB + gs) * B + c

    idx_host = np.zeros((NC, NS, P, SLAB * B), np.int32)
    dst_host = np.zeros((NC, NS, P, SLAB * B), np.float32)
    ea_host = np.zeros((NC, NS, P, SLAB * B * A), np.float32)
    idx_host.reshape(-1)[pos] = src[perm].astype(np.int32)
    dst_host.reshape(-1)[pos] = (dsts & (P - 1)).astype(np.float32)
    ea_host.reshape(-1, A)[pos] = ea[perm]

    w2 = np.ascontiguousarray(
        np.asarray(weight_matrix, dtype=np.float32).transpose(0, 2, 1)
    ).reshape(A * CIN, COUT)                                # [(a,i), o]
    bias_t = np.ascontiguousarray(
        np.broadcast_to(np.asarray(bias, dtype=np.float32).reshape(1, COUT),
                        (P, COUT))
    )
    return x, idx_host, dst_host, ea_host, w2, bias_t


# --------------------------------------------------------------------------
# device kernel
# --------------------------------------------------------------------------

def build_bass(cfg, use_bf16=True):
    import concourse.bacc as bacc
    import concourse.bass as bass
    import concourse.mybir as mybir
    import concourse.tile as tile
    from concourse._compat import axon_active

    f32 = mybir.dt.float32
    bf16 = mybir.dt.bfloat16
    i32 = mybir.dt.int32
    # compute dtype for z / onehot / matmul: "bf16" flag now means fp16 —
    # same PE/DVE speed tier as bf16 but 11 mantissa bits (~5e-4 rel)
    cdt = mybir.dt.float16 if use_bf16 else f32

    B = cfg.cpg
    SB = cfg.sb_chunks      # chunks per slab
    SLAB = cfg.slab_groups

    nc = bacc.Bacc(
        "TRN2",
        target_bir_lowering=False,
        debug=False,
        enable_asserts=False,
        num_devices=cfg.n_cores,
    )

    x_d = nc.dram_tensor("x", [cfg.n_nodes, CIN], f32, kind="ExternalInput")
    ea_d = nc.dram_tensor("ea", [cfg.nslabs, P, SB * A], f32, kind="ExternalInput")
    idx_d = nc.dram_tensor("idx", [cfg.nslabs, P, SB], i32, kind="ExternalInput")
    dst_d = nc.dram_tensor("dstl", [cfg.nslabs, P, SB], f32, kind="ExternalInput")
    w2_d = nc.dram_tensor("w2", [A * CIN, COUT], f32, kind="ExternalInput")
    b_d = nc.dram_tensor("bias", [P, COUT], f32, kind="ExternalInput")
    out_d = nc.dram_tensor(
        "out", [cfg.nslabs, SLAB, P, COUT], f32, kind="ExternalOutput"
    )

    with tile.TileContext(nc) as tc:
        with (
            tc.tile_pool(name="const", bufs=1) as cpool,
            tc.tile_pool(name="slab_in", bufs=2) as spool,
            tc.tile_pool(name="xj", bufs=2) as xjpool,
            tc.tile_pool(name="zoh", bufs=3) as zpool,
            tc.tile_pool(name="q", bufs=2) as qpool,
            tc.tile_pool(name="ostage", bufs=2) as opool,
            tc.tile_pool(name="psq", bufs=3, space="PSUM") as psq,
            tc.tile_pool(name="pso", bufs=2, space="PSUM") as pso,
        ):
            # constants
            iota_t = cpool.tile([P, P], cdt, tag="iota")
            nc.gpsimd.iota(iota_t[:], pattern=[[1, P]], base=0,
                           channel_multiplier=0,
                           allow_small_or_imprecise_dtypes=True)
            w2_t = cpool.tile([A * CIN, COUT], f32, tag="w2")
            nc.sync.dma_start(out=w2_t[:], in_=w2_d.ap())
            bias_t = cpool.tile([P, COUT], f32, tag="bias")
            nc.sync.dma_start(out=bias_t[:], in_=b_d.ap())

            for s in range(cfg.nslabs):
                idx_t = spool.tile([P, SB], i32, tag="idx")
                nc.sync.dma_start(out=idx_t[:], in_=idx_d.ap()[s])
                dst_t = spool.tile([P, SB], cdt, tag="dst")
                if use_bf16:
                    # values are 0..127: exact in bf16; SWDGE casts in flight
                    nc.gpsimd.dma_start(out=dst_t[:], in_=dst_d.ap()[s])
                else:
                    nc.sync.dma_start(out=dst_t[:], in_=dst_d.ap()[s])
                if use_bf16:
                    ea_t = spool.tile([P, SB * A], bf16, tag="ea")
                    # SWDGE casts f32 -> bf16 during the copy
                    nc.gpsimd.dma_start(out=ea_t[:], in_=ea_d.ap()[s])
                else:
                    ea_t = spool.tile([P, SB * A], f32, tag="ea")
                    nc.sync.dma_start(out=ea_t[:], in_=ea_d.ap()[s])

                # indirect gather: HW consumes exactly ONE index per
                # partition per instruction (per-partition dynamic block
                # copy), so issue one gather per 128-edge chunk.
                xj_t = xjpool.tile([P, SB * CIN], f32, tag="xj")
                for c in range(SB):
                    nc.gpsimd.indirect_dma_start(
                        out=xj_t[:, c * CIN:(c + 1) * CIN],
                        out_offset=None,
                        in_=x_d.ap(),
                        in_offset=bass.IndirectOffsetOnAxis(
                            ap=idx_t[:, c:c + 1], axis=0),
                    )
                if use_bf16:
                    xjc_t = xjpool.tile([P, SB * CIN], bf16, tag="xjc")
                    nc.vector.tensor_copy(out=xjc_t[:], in_=xj_t[:])
                else:
                    xjc_t = xj_t

                out_sb = opool.tile([P, SLAB * COUT], f32, tag="ostage")

                for gs in range(SLAB):
                    # z[e, (c, a, i)] = ea[e, c, a] * xj[e, c, i]
                    z_t = zpool.tile([P, B * P], cdt, tag="z")
                    ea_ap = (
                        ea_t[:, gs * B * A:(gs + 1) * B * A]
                        .rearrange("p (b a) -> p b a", a=A)
                        .unsqueeze(3)
                        .to_broadcast([P, B, A, CIN])
                    )
                    xj_ap = (
                        xjc_t[:, gs * B * CIN:(gs + 1) * B * CIN]
                        .rearrange("p (b i) -> p b i", i=CIN)
                        .unsqueeze(2)
                        .to_broadcast([P, B, A, CIN])
                    )
                    z_ap = z_t[:].rearrange("p (b a i) -> p b a i", a=A, i=CIN)
                    nc.vector.tensor_tensor(
                        out=z_ap, in0=ea_ap, in1=xj_ap, op=mybir.AluOpType.mult
                    )

                    # onehot[e, (c, n)] = (dst_local[e, c] == n)
                    oh_t = zpool.tile([P, B * P], cdt, tag="oh")
                    iota_ap = (
                        iota_t[:].unsqueeze(1).to_broadcast([P, B, P])
                    )
                    dstg_ap = (
                        dst_t[:, gs * B:(gs + 1) * B]
                        .unsqueeze(2)
                        .to_broadcast([P, B, P])
                    )
                    oh_ap = oh_t[:].rearrange("p (b n) -> p b n", n=P)
                    nc.vector.tensor_tensor(
                        out=oh_ap, in0=iota_ap, in1=dstg_ap,
                        op=mybir.AluOpType.is_equal,
                    )

                    # Q_T[(a,i), n] += z.T @ onehot     (accumulate B chunks)
                    q_ps = psq.tile([P, P], f32, tag="qps")
                    for c in range(B):
                        nc.tensor.matmul(
                            out=q_ps[:],
                            lhsT=z_t[:, c * P:(c + 1) * P],
                            rhs=oh_t[:, c * P:(c + 1) * P],
                            start=(c == 0),
                            stop=(c == B - 1),
                        )
                    q_sb = qpool.tile([P, P], f32, tag="qsb")
                    nc.scalar.activation(
                        out=q_sb[:], in_=q_ps[:],
                        func=mybir.ActivationFunctionType.Copy,
                    )

                    # aggr = Q_T.T @ W2   -> [128n, 16]
                    o_ps = pso.tile([P, COUT], f32, tag="ops")
                    nc.tensor.matmul(
                        out=o_ps[:], lhsT=q_sb[:], rhs=w2_t[:],
                        start=True, stop=True,
                    )
                    # relu(aggr + bias): add bias, then clamp at 0 in place
                    oslice = out_sb[:, gs * COUT:(gs + 1) * COUT]
                    nc.vector.tensor_tensor(
                        out=oslice, in0=o_ps[:], in1=bias_t[:],
                        op=mybir.AluOpType.add,
                    )
                    nc.vector.tensor_scalar(
                        out=oslice, in0=oslice, scalar1=0.0, scalar2=None,
                        op0=mybir.AluOpType.max,
                    )

                # store the slab: SBUF [128, SLAB*16] -> DRAM [SLAB, 128, 16]
                nc.sync.dma_start(
                    out=out_d.ap()[s].transpose([1, 0, 2]),
                    in_=out_sb[:].rearrange("p (g o) -> p g o", o=COUT),
                )

    nc.compile()
    return nc


# --------------------------------------------------------------------------
# runner
# --------------------------------------------------------------------------

_CACHE = {}


def _get_compiled(cfg, use_bf16):
    key = (cfg.n_nodes, cfg.n_edges, cfg.cpg, use_bf16)
    if key not in _CACHE:
        _CACHE[key] = build_bass(cfg, use_bf16=use_bf16)
    return _CACHE[key]


def kernel(x, edge_index, edge_attr, weight_matrix, bias, num_nodes):
    import time as _time
    cfg = Cfg(n_nodes=FULL.n_nodes, n_edges=FULL.n_edges)
    assert int(num_nodes) == cfg.n_nodes
    use_bf16 = os.environ.get("GNN_BF16", "0") == "1"

    _t0 = _time.time()
    xf, idx_h, dst_h, ea_h, w2, bias_f = host_prep(
        cfg, x, edge_index, edge_attr, weight_matrix, bias
    )
    _t1 = _time.time()
    nc = _get_compiled(cfg, use_bf16)
    _t2 = _time.time()

    in_maps = []
    for k in range(cfg.n_cores):
        in_maps.append({
            "x": xf,
            "ea": ea_h[k],
            "idx": idx_h[k],
            "dstl": dst_h[k],
            "w2": w2,
            "bias": bias_f,
        })

    from concourse import bass_utils
    res = bass_utils.run_bass_kernel_spmd(
        nc, in_maps, core_ids=list(range(cfg.n_cores)),
        trace=os.environ.get("GNN_TRACE", "0") == "1",
    )
    _t3 = _time.time()
    print(f"[kernel] host_prep {_t1 - _t0:.2f}s  compile {_t2 - _t1:.2f}s  "
          f"device-run {_t3 - _t2:.2f}s")
    outs = [r["out"].reshape(-1, COUT) for r in res.results]
    full = np.concatenate(outs, axis=0)[:cfg.n_nodes]
    kernel.last_results = res
    return np.ascontiguousarray(full, dtype=np.float32)


kernel.last_results = None
